# revision 42
# baseline (speedup 1.0000x reference)
"""Trainium2 Bass kernel: causal self-attention with RoPE.

Model (matches the reference nn.Module):
    B=4, T=2048, C=1024, H=16 heads, head_dim=64
    qkv = x @ W_attn + b_attn ; rope(q, k) ; causal softmax(q k^T / 8) @ v
    out = y @ W_proj + b_proj

Sharding over 8 NeuronCores: data parallel on batch (4) x tensor parallel on
heads (2 groups of 8). Each core computes its batch's 8 heads end to end and
a partial y @ W_proj over its 512 head-dims; the host sums the two partial
projections per batch and adds b_proj.

Everything on-chip stays in "feature on partitions" (transposed) layout so
every matmul contracts over the partition dim with zero transposes:
  x^T [C,T] -> K^T [512,T] resident / Q^T per 512-query stripe (RoPE's
  rotate-half realized as a PE permutation matmul + two table multiplies,
  signs folded into the sin table), V [T,512] natural with a ones column per
  head (the softmax denominator falls out of the same matmul that
  accumulates att @ V; diagonal blocks restrict to the causal q-range).

All SBUF operands are bf16 (PSUM accumulation and the final output stay
f32): bf16 matmuls run 1 cycle/row at every free-dim size, DVE element-wise
ops get the 2-4x packed-dtype modes, and DMA traffic halves.  The attention
inner loop is software-pipelined with att@V skewed two slots behind the
score matmuls so the PE never waits on the scalar-engine exp; projection for
stripe g runs as PE filler inside the (ACT-lean) attention of stripe g+1.
"""

import os
import sys
from contextlib import ExitStack

for _p in ("/opt/trn_rl_repo", "/root/.axon_site/_ro/trn_rl_repo"):
    if os.path.isdir(_p) and _p not in sys.path:
        sys.path.append(_p)

import numpy as np
import ml_dtypes

import bass_rust
import concourse.bass as bass
import concourse.mybir as mybir
from concourse import tile
from concourse.bass_utils import run_bass_kernel_spmd

F32 = mybir.dt.float32
BF16 = mybir.dt.bfloat16
FP8 = mybir.dt.float8e4
Act = mybir.ActivationFunctionType
DR = mybir.MatmulPerfMode.DoubleRow

B, T, C = 4, 2048, 1024
H, HD = 16, 64
HL = 8          # heads per core
N_CORES = 8
ROPE_BASE = 10000.0

T8 = 256        # t slice width for the qkv phase
QB = 512        # query stripe width
KB = 128        # key block for attention
NKB = T // KB   # 16
NQG = T // QB   # 4


def split_excess_waits(nc, max_waits=1):
    """The walrus build in this container supports only one sync-wait command
    per instruction (all engine templates); hoist extra semaphore waits onto
    same-engine NoOps inserted immediately before the instruction (same
    engine timeline, so semantics are unchanged)."""
    ctr = 0
    for fn in nc.m.functions:
        for blk in fn.blocks:
            new_insts = []
            changed = False
            for inst in blk.instructions:
                si = inst.sync_info
                if si is not None:
                    waits = list(si.on_wait)
                    sem_waits = [w for w in waits if w.sync_type == "semaphore"]
                    other = [w for w in waits if w.sync_type != "semaphore"]
                    budget = max(0, max_waits - len(other))
                    if len(sem_waits) > budget:
                        keep = sem_waits[:budget]
                        extra = sem_waits[budget:]
                        step = max(1, max_waits)
                        for i in range(0, len(extra), step):
                            nop = bass_rust.InstNoOp(
                                name=f"WSPLIT-{ctr}", ins=[], outs=[])
                            ctr += 1
                            nop.engine = inst.engine
                            nop.sync_info = bass_rust.SyncInfo(
                                on_wait=extra[i:i + step], on_update=[])
                            new_insts.append(nop)
                        si.on_wait = other + keep
                        changed = True
                new_insts.append(inst)
            if changed:
                blk.instructions = new_insts


def build_nc(split=True):
    nc = bass.Bass("TRN2", target_bir_lowering=False, debug=False,
                   num_devices=N_CORES)

    xT_d = nc.dram_tensor("xT", [C, T], BF16, kind="ExternalInput")
    wq_d = nc.dram_tensor("wq", [C, 512], BF16, kind="ExternalInput")
    wk_d = nc.dram_tensor("wk", [C, 512], BF16, kind="ExternalInput")
    wv_d = nc.dram_tensor("wv", [C, 512], BF16, kind="ExternalInput")
    wp_d = nc.dram_tensor("wp", [512, C], BF16, kind="ExternalInput")
    bq_d = nc.dram_tensor("bq", [512], F32, kind="ExternalInput")
    bk_d = nc.dram_tensor("bk", [512], F32, kind="ExternalInput")
    bv_d = nc.dram_tensor("bvrep", [128, 512], F32, kind="ExternalInput")
    cos_d = nc.dram_tensor("cos128", [128, T], BF16, kind="ExternalInput")
    sin_d = nc.dram_tensor("sin128", [128, T], F32, kind="ExternalInput")
    mask_d = nc.dram_tensor("masks", [4, 128, QB], BF16, kind="ExternalInput")
    ones_d = nc.dram_tensor("ones128", [128, 128], BF16, kind="ExternalInput")
    sperm_d = nc.dram_tensor("sperm", [128, 128], BF16, kind="ExternalInput")
    outT_d = nc.dram_tensor("outT", [C, T], BF16, kind="ExternalOutput")

    with tile.TileContext(nc) as tc, ExitStack() as ctx:
        const = ctx.enter_context(tc.tile_pool(name="const", bufs=1))
        persist = ctx.enter_context(tc.tile_pool(name="persist", bufs=1))
        wres = ctx.enter_context(tc.tile_pool(name="wres", bufs=1))
        cs_pool = ctx.enter_context(tc.tile_pool(name="cs_pool", bufs=2))
        xt_pool = ctx.enter_context(tc.tile_pool(name="xt_pool", bufs=2))
        qts_pool = ctx.enter_context(tc.tile_pool(name="qts_pool", bufs=2))
        aux_pool = ctx.enter_context(tc.tile_pool(name="aux_pool", bufs=2))
        pt_pool = ctx.enter_context(tc.tile_pool(name="pt_pool", bufs=5))
        misc_pool = ctx.enter_context(tc.tile_pool(name="misc_pool", bufs=2))
        yt_pool = ctx.enter_context(tc.tile_pool(name="yt_pool", bufs=3))
        out_pool = ctx.enter_context(tc.tile_pool(name="out_pool", bufs=4))
        s_pool = ctx.enter_context(
            tc.tile_pool(name="s_pool", bufs=2, space="PSUM"))
        sc_pool = ctx.enter_context(
            tc.tile_pool(name="sc_pool", bufs=2, space="PSUM"))
        o_pool = ctx.enter_context(
            tc.tile_pool(name="o_pool", bufs=1, space="PSUM"))

        # ---- constants / weights: tiles declared up front, DMAs emitted
        # just before first use so early queues prioritize the critical path
        ones_sb = const.tile([128, 128], BF16, tag="ones", name="ones_sb")
        sperm_sb = const.tile([128, 128], BF16, tag="sperm", name="sperm_sb")
        bq_sb = const.tile([128, 4], F32, tag="bq", name="bq_sb")
        bk_sb = const.tile([128, 4], F32, tag="bk", name="bk_sb")
        bv_sb = const.tile([128, 512], F32, tag="bv", name="bv_sb")
        mask_sb = const.tile([128, 4 * QB], BF16, tag="mask", name="mask_sb")

        kt_t = [persist.tile([128, T], BF16, tag=f"kt{i}", name=f"kt{i}")
                for i in range(4)]
        v_sb = persist.tile([128, HL * NKB * 65], BF16, tag="v", name="v_sb")

        wq_sb = [wres.tile([128, 4 * 512], BF16, tag=f"wq{i}",
                           name=f"wq_sb{i}") for i in range(2)]
        wk_sb = wres.tile([128, 8 * 512], BF16, tag="wk", name="wk_sb")
        wv_sb = wres.tile([128, 8 * 512], BF16, tag="wv", name="wv_sb")
        wp_sb = wres.tile([128, 4 * C], BF16, tag="wp", name="wp_sb")

        # ---- emission as unit closures so next-stripe QKV and prev-stripe
        # projection interleave into the ACT-bound attention loop
        stripe_state = {}

        def qkv_units(g):
            st = {}
            stripe_state[g] = st
            gs, ge = g * QB, (g + 1) * QB
            units = []       # Q path: alloc, x loads, Q chunks, rope-Q
            kv_units = []    # K/V chunks (emitted after the Q path)
            ropek_units = []

            def u_alloc():
                st["qts"] = [qts_pool.tile([128, QB], BF16, tag=f"qts{mc}",
                                           name=f"qts{mc}_{g}")
                             for mc in range(4)]
                st["cosS"] = cs_pool.tile([128, QB], BF16, tag="cosS",
                                          name=f"cosS{g}")
                st["sinS"] = cs_pool.tile([128, QB], F32, tag="sinS",
                                          name=f"sinS{g}")
            units.append(u_alloc)

            for t8l in range(2):
                t8 = 2 * g + t8l
                ts, te = t8 * T8, (t8 + 1) * T8

                def u_load(t8=t8, t8l=t8l, ts=ts, te=te):
                    xt = xt_pool.tile([128, 8 * T8], BF16, tag="xt",
                                      name=f"xt{t8}")
                    st["xt", t8l] = xt
                    xv = xt[:].rearrange("p (cc t) -> p cc t", cc=8)
                    for half, eng in enumerate((nc.sync, nc.scalar)):
                        eng.dma_start(
                            xv[:, half * 4:(half + 1) * 4],
                            xT_d[512 * half:512 * (half + 1), ts:te]
                            .rearrange("(cc p) t -> p cc t", p=128))
                units.append(u_load)

                for is_q in (True, False):
                    for mc in range(4):
                        def u_qk(t8=t8, t8l=t8l, ts=ts, te=te,
                                 is_q=is_q, mc=mc):
                            bias_sb = bq_sb if is_q else bk_sb
                            xt = st["xt", t8l]
                            ps = sc_pool.tile([128, 512], F32, tag="sc",
                                              name=f"ps{t8}_{mc}_{int(is_q)}")
                            for cc in range(8):
                                if is_q:
                                    wsb = wq_sb[cc // 4]
                                    wcol = (cc % 4) * 512 + mc * 128
                                else:
                                    wsb = wk_sb
                                    wcol = cc * 512 + mc * 128
                                nc.tensor.matmul(
                                    ps[:, 0:T8],
                                    lhsT=wsb[:, wcol:wcol + 128],
                                    rhs=xt[:, cc * T8:(cc + 1) * T8],
                                    start=(cc == 0), stop=(cc == 7))
                            if is_q:
                                dst = st["qts"][mc][:, t8l * T8:
                                                    (t8l + 1) * T8]
                            else:
                                dst = kt_t[mc][:, ts:te]
                            nc.vector.tensor_scalar_add(dst, ps[:, 0:T8],
                                                        bias_sb[:, mc:mc + 1])
                        (units if is_q else kv_units).append(u_qk)

                for tbl in range(T8 // 128):
                    def u_v(t8=t8, t8l=t8l, tbl=tbl):
                        tb = t8 * (T8 // 128) + tbl
                        xt = st["xt", t8l]
                        ps = sc_pool.tile([128, 512], F32, tag="sc",
                                          name=f"psv{t8}_{tbl}")
                        for cc in range(8):
                            nc.tensor.matmul(
                                ps[:],
                                lhsT=xt[:, cc * T8 + tbl * 128:
                                        cc * T8 + tbl * 128 + 128],
                                rhs=wv_sb[:, cc * 512:(cc + 1) * 512],
                                start=(cc == 0), stop=(cc == 7))
                        nc.vector.tensor_add(
                            v_sb[:].rearrange("p (h t c) -> p h t c",
                                              h=HL, c=65)[:, :, tb, 0:64],
                            ps[:].rearrange("p (h c) -> p h c", h=HL),
                            bv_sb[:].rearrange("p (h c) -> p h c", h=HL))
                    kv_units.append(u_v)

            def u_cs():
                nc.scalar.dma_start(st["cosS"][:], cos_d[:, gs:ge])
                nc.scalar.dma_start(st["sinS"][:], sin_d[:, gs:ge])
            units.append(u_cs)
            for is_q in (True, False):
                for mc in range(4):
                    def u_rope(is_q=is_q, mc=mc):
                        dst = (st["qts"][mc][:] if is_q
                               else kt_t[mc][:, gs:ge])
                        aux_ps = sc_pool.tile([128, QB], F32, tag="sc",
                                              name=f"axp{g}_{mc}_{int(is_q)}")
                        nc.tensor.matmul(aux_ps[:], lhsT=sperm_sb[:],
                                         rhs=dst, start=True, stop=True)
                        aux = aux_pool.tile([128, QB], BF16, tag="aux",
                                            name=f"aux{g}_{mc}_{int(is_q)}")
                        nc.vector.tensor_mul(aux[:], aux_ps[:], st["sinS"][:])
                        nc.gpsimd.tensor_mul(dst, dst, st["cosS"][:])
                        nc.vector.tensor_add(dst, dst, aux[:])
                    (units if is_q else ropek_units).append(u_rope)
            if g == 0:
                def u_specials():
                    for cc, eng in enumerate((nc.sync, nc.scalar)):
                        eng.dma_start(
                            wk_sb[:].rearrange("p (cc m) -> p cc m",
                                               cc=8)[:, 4 * cc:4 * cc + 4],
                            wk_d[512 * cc:512 * (cc + 1), :]
                            .rearrange("(cc p) m -> p cc m", p=128))
                    nc.sync.dma_start(
                        bk_sb[:], bk_d.rearrange("(m p) -> p m", p=128))
                    for cc, eng in enumerate((nc.sync, nc.scalar)):
                        eng.dma_start(
                            wv_sb[:].rearrange("p (cc m) -> p cc m",
                                               cc=8)[:, 4 * cc:4 * cc + 4],
                            wv_d[512 * cc:512 * (cc + 1), :]
                            .rearrange("(cc p) m -> p cc m", p=128))
                    nc.gpsimd.dma_start(ones_sb[:], ones_d[:])
                    nc.gpsimd.dma_start(bv_sb[:], bv_d[:])
                    nc.sync.dma_start(
                        v_sb[:].rearrange("p (blk c) -> p blk c",
                                          c=65)[:, :, 64:65],
                        ones_d[:].rearrange("p (b o) -> p b o", o=1))
                kv_units.insert(0, u_specials)
            return units, kv_units, ropek_units

        def attn_units(g):
            st = stripe_state[g]
            units = []
            nkb = 4 * g + 4

            def qlo_of(kb):
                r = kb - 4 * g if kb >= 4 * g else None
                return r, (r * KB if r else 0)

            for hp in range(4):
                # software-pipelined chain: slot k runs score(k) on the PE,
                # exp(k) on ACT, and attV(k-2) on the PE, so the attV never
                # waits on the exp of its own block
                for k in range(nkb + 2):
                    def u_slot(hp=hp, k=k):
                        qts = st["qts"]
                        if k == 0:
                            st["o", hp] = [
                                o_pool.tile([65, 512], F32, tag=f"o{hh}",
                                            name=f"o{hh}_{g}_{hp}")
                                for hh in range(2)]
                        o_ps = st["o", hp]
                        if k < nkb:
                            kb = k
                            r, qlo = qlo_of(kb)
                            s_ps = s_pool.tile([128, 2 * QB], F32, tag="s",
                                               name=f"s_{g}_{hp}_{kb}")
                            for hh in range(2):
                                nc.tensor.matmul(
                                    s_ps[:, hh * QB + qlo:(hh + 1) * QB],
                                    lhsT=kt_t[hp][hh * 64:(hh + 1) * 64,
                                                  kb * KB:(kb + 1) * KB],
                                    rhs=qts[hp][hh * 64:(hh + 1) * 64, qlo:],
                                    start=True, stop=True,
                                    tile_position=(hh * 64, 0))
                            pt = pt_pool.tile([128, 2 * QB], BF16, tag="pt",
                                              name=f"pt_{g}_{hp}_{kb}")
                            st["pt", hp, kb] = pt
                            if qlo == 0:
                                nc.scalar.activation(pt[:], s_ps[:], Act.Exp,
                                                     scale=0.125)
                            else:
                                # both heads' causal ranges in one 3D-AP op
                                nc.scalar.activation(
                                    pt[:].rearrange("p (h q) -> p h q",
                                                    h=2)[:, :, qlo:],
                                    s_ps[:].rearrange("p (h q) -> p h q",
                                                      h=2)[:, :, qlo:],
                                    Act.Exp, scale=0.125)
                            if r is not None:
                                # only the 128-wide diagonal block is
                                # actually partial; the rest is all-ones
                                for hh in range(2):
                                    nc.vector.tensor_mul(
                                        pt[:, hh * QB + qlo:
                                           hh * QB + qlo + KB],
                                        pt[:, hh * QB + qlo:
                                           hh * QB + qlo + KB],
                                        mask_sb[:, r * QB + qlo:
                                                r * QB + qlo + KB])
                        if k >= 2:
                            kb2 = k - 2
                            r2, qlo2 = qlo_of(kb2)
                            pt2 = st.pop(("pt", hp, kb2))
                            for hh in range(2):
                                h = hp * 2 + hh
                                off = (h * NKB + kb2) * 65
                                nc.tensor.matmul(
                                    o_ps[hh][:, qlo2:],
                                    lhsT=v_sb[:, off:off + 65],
                                    rhs=pt2[:, hh * QB + qlo2:
                                            (hh + 1) * QB],
                                    start=(kb2 == 0), stop=(kb2 == nkb - 1))
                    units.append(u_slot)

                for hh in range(2):
                    def u_div(hp=hp, hh=hh):
                        if hp == 0 and hh == 0:
                            st["yts"] = [
                                yt_pool.tile([128, QB], BF16, tag=f"yt{i}",
                                             name=f"yt{i}_{g}")
                                for i in range(4)]
                        o_ps = st["o", hp]
                        # one copy of all 65 rows frees the PSUM o tile as
                        # early as possible for the next hp's accumulation;
                        # at the kernel tail ACT is idle, so let it take the
                        # second head's copy off the DVE chain
                        o65 = misc_pool.tile([65, 512], F32, tag="o65",
                                             name=f"ob_{g}_{hp}_{hh}")
                        if g == NQG - 1 and hp == 3 and hh == 1:
                            nc.scalar.copy(o65[:], o_ps[hh][:])
                        else:
                            nc.vector.tensor_copy(o65[:], o_ps[hh][:])
                        recip = misc_pool.tile([65, 512], BF16, tag="recip",
                                               name=f"rc_{g}_{hp}_{hh}")
                        with nc.allow_low_precision(
                                reason="bf16 softmax denominators"):
                            nc.vector.reciprocal(recip[64:65, :],
                                                 o65[64:65, :])
                        if hh == 0:
                            st["b2", hp] = sc_pool.tile(
                                [128, 512], F32, tag="sc",
                                name=f"b_{g}_{hp}")
                        b2 = st["b2", hp]
                        nc.tensor.matmul(b2[hh * 64:(hh + 1) * 64, :],
                                         lhsT=ones_sb[64:65, 0:64],
                                         rhs=recip[64:65, :],
                                         start=True, stop=True,
                                         tile_position=(64, hh * 64))
                        nc.vector.tensor_mul(
                            st["yts"][hp][hh * 64:(hh + 1) * 64, :],
                            o65[0:64, :], b2[hh * 64:(hh + 1) * 64, :])
                    units.append(u_div)
            return units

        def proj_units(g):
            st = stripe_state[g]
            units = []
            if g == 0:
                def u_wp():
                    for cc in range(4):
                        nc.gpsimd.dma_start(wp_sb[:, cc * C:(cc + 1) * C],
                                            wp_d[cc * 128:(cc + 1) * 128, :])
                units.append(u_wp)
            for co in range(8):
                def u_proj(co=co):
                    yts = st["yts"]
                    ps = sc_pool.tile([128, 512], F32, tag="sc",
                                      name=f"pps_{g}_{co}")
                    for cc in range(4):
                        nc.tensor.matmul(
                            ps[:],
                            lhsT=wp_sb[:, cc * C + co * 128:
                                       cc * C + (co + 1) * 128],
                            rhs=yts[cc][:],
                            start=(cc == 0), stop=(cc == 3))
                    osb = out_pool.tile([128, 512], BF16, tag="out",
                                        name=f"out_{g}_{co}")
                    if g == NQG - 1:
                        # ACT is exp-free at the tail; alternate with DVE so
                        # the final copies don't serialize on one engine
                        if co % 2 == 0:
                            nc.scalar.copy(osb[:], ps[:])
                        else:
                            nc.vector.tensor_copy(osb[:], ps[:])
                    else:
                        nc.vector.tensor_copy(osb[:], ps[:])
                    nc.sync.dma_start(
                        outT_d[co * 128:(co + 1) * 128,
                               g * QB:(g + 1) * QB],
                        osb[:])
                units.append(u_proj)
            return units

        def interleave(main, fill, boundaries):
            """Emit `main` units; at each index in `boundaries` (fraction of
            main consumed) flush the proportional share of `fill`."""
            n, m = len(main), len(fill)
            fi = 0
            cut = {int(b * n): True for b in boundaries}
            for i, u in enumerate(main):
                u()
                if i + 1 in cut or i + 1 == n:
                    want = ((i + 1) * m) // n
                    while fi < want:
                        fill[fi]()
                        fi += 1
            while fi < m:
                fill[fi]()
                fi += 1

        q0, kv0, rk0 = qkv_units(0)
        for u in q0[:2]:
            u()
        # Q weights right behind the first x slice; biases + rope perm after
        for cc, eng in enumerate((nc.sync, nc.scalar)):
            eng.dma_start(
                wq_sb[cc][:].rearrange("p (cc m) -> p cc m", cc=4),
                wq_d[512 * cc:512 * (cc + 1), :]
                .rearrange("(cc p) m -> p cc m", p=128))
        nc.sync.dma_start(bq_sb[:], bq_d.rearrange("(m p) -> p m", p=128))
        nc.sync.dma_start(sperm_sb[:], sperm_d[:])
        for u in q0[2:]:
            u()
        # causal masks gate the very first attention block; load them on the
        # fast HWDGE queue (behind the rope tables) rather than SWDGE
        nc.scalar.dma_start(
            mask_sb[:].rearrange("p (r q) -> p r q", r=4),
            mask_d.rearrange("r p q -> p r q"))
        for u in kv0 + rk0:
            u()
        for g in range(NQG):
            if g + 1 < NQG:
                qp, kv, rk = qkv_units(g + 1)
            else:
                qp, kv, rk = [], [], []
            main = attn_units(g)
            # prev-stripe projection leads (its deps resolve first), then
            # next-stripe QKV.  proj(1) and proj(2) are held back for the
            # last stripe, whose attention is otherwise ACT-bound (nothing
            # else left to feed the PE).
            if g == 1:
                pfill = proj_units(0)
            elif g == 3:
                pfill = proj_units(1) + proj_units(2)
            else:
                pfill = []
            fill = pfill + qp + kv + rk
            if g == NQG - 1 and len(fill) >= 4:
                # hold filler units back so the PE has work while the
                # last head-pair's softmax-divide chain runs on the DVE
                main, tail = main[:-2], main[-2:]
                fill, resv = fill[:-4], fill[-4:]
            else:
                tail, resv = [], []
            interleave(main, fill,
                       tuple(i / len(main) for i in range(1, len(main))))
            for u in resv + tail:
                u()
        for u in proj_units(NQG - 1):
            u()

    if split:
        split_excess_waits(nc)
    return nc


_NC = None


def _get_nc():
    global _NC
    if _NC is None:
        _NC = build_nc()
    return _NC


def _rope_tables_128():
    rot = HD // 2  # 32
    inv_freq = 1.0 / (ROPE_BASE ** (np.arange(0, rot, 2, dtype=np.float32)
                                    / np.float32(rot)))
    pos = np.arange(T, dtype=np.float32)
    freqs = np.outer(pos, inv_freq).astype(np.float32)   # [T, 16]
    emb = np.concatenate([freqs, freqs], axis=-1)        # [T, 32]
    cosT = np.cos(emb).astype(np.float32).T              # [32, T]
    sinT = np.sin(emb).astype(np.float32).T
    cos128 = np.ascontiguousarray(np.tile(cosT, (4, 1)))
    sgn = np.ones((128, 1), np.float32)
    sgn[0:32] = -1.0
    sgn[64:96] = -1.0
    sin128 = np.ascontiguousarray(np.tile(sinT, (4, 1)) * sgn)
    return cos128, sin128


def _sperm():
    # permutation: aux[m] = dst[swap(m)], swap exchanges 32-halves in each
    # 64-row head block (sign handled by the sin table)
    P = np.zeros((128, 128), np.float32)
    for m in range(128):
        blk, r = m // 64, m % 64
        k = blk * 64 + (r + 32) % 64
        P[k, m] = 1.0
    return P


def _masks():
    kp = np.arange(128, dtype=np.int64)[:, None]
    qf = np.arange(QB, dtype=np.int64)[None, :]
    out = np.empty((4, 128, QB), np.float32)
    for r in range(4):
        out[r] = ((r * KB + kp) <= qf).astype(np.float32)
    return out


def _bf16(a):
    return np.ascontiguousarray(a.astype(ml_dtypes.bfloat16))


def _in_maps(x, W_attn, b_attn, W_proj):
    cos128, sin128 = _rope_tables_128()
    masks = _bf16(_masks())
    ones = _bf16(np.ones((128, 128), np.float32))
    sperm = _bf16(_sperm())
    cos128 = _bf16(cos128)
    maps = []
    for c in range(N_CORES):
        b, hg = c // 2, c % 2
        sl = slice(hg * 512, (hg + 1) * 512)
        maps.append({
            "xT": _bf16(x[b].T),
            "wq": _bf16(W_attn[:, 0 * C:1 * C][:, sl]),
            "wk": _bf16(W_attn[:, 1 * C:2 * C][:, sl]),
            "wv": _bf16(W_attn[:, 2 * C:3 * C][:, sl]),
            "wp": _bf16(W_proj[sl, :]),
            "bq": np.ascontiguousarray(b_attn[0 * C:1 * C][sl]),
            "bk": np.ascontiguousarray(b_attn[1 * C:2 * C][sl]),
            "bvrep": np.ascontiguousarray(
                np.broadcast_to(b_attn[2 * C:3 * C][sl], (128, 512))),
            "sperm": sperm,
            "cos128": cos128,
            "sin128": sin128,
            "masks": masks,
            "ones128": ones,
        })
    return maps


def kernel(x, W_attn, b_attn, W_proj, b_proj):
    x = np.asarray(x, dtype=np.float32)
    W_attn = np.asarray(W_attn, dtype=np.float32)
    b_attn = np.asarray(b_attn, dtype=np.float32)
    W_proj = np.asarray(W_proj, dtype=np.float32)
    b_proj = np.asarray(b_proj, dtype=np.float32)

    nc = _get_nc()
    maps = _in_maps(x, W_attn, b_attn, W_proj)
    res = run_bass_kernel_spmd(nc, maps, list(range(N_CORES)))

    out = np.empty((B, T, C), np.float32)
    for b in range(B):
        acc = (res.results[2 * b]["outT"].astype(np.float32)
               + res.results[2 * b + 1]["outT"].astype(np.float32))
        out[b] = acc.T + b_proj[None, :]
    return out
